# revision 14
# baseline (speedup 1.0000x reference)
"""DUQ RBF head kernel for Trainium2 (8 NeuronCores, batch-parallel).

Computes out[b,c,h,w] = exp(gamma * mean_e (einsum('bfhw,ecf', x, W) - m/N)^2)
for features [8,512,128,128], weights [16,64,512], m [16,64], N [64].

Strategy: data-parallel over batch (1 image per core). Per core, one big
matmul [ec=1024, f=512] @ [f=512, pix=16384] in float32r (full-rate fp32,
self-loading weights; LDWEIGHTS hides behind the previous matmul's
streaming; every matmul carries a fixed ~13.5ns issue overhead on top of
N/2.4GHz streaming, so N=512 everywhere it can be). Pixels are processed
in groups of up to 1024 (two 512-col PSUM banks per ec-chunk); the Square
epilogue folds the centroid into the per-partition ACT bias, DVE
accumulates the 8 squared ec-chunks and folds the two e-halves, ACT
applies the final Exp. N per matmul stays >= 256: fp32r is 1/4-rate
below 256 moving columns.

Startup (the schedulable slack; steady state is PE-streaming-bound):
- The first ~2MB of weights+x is HBM/ring-limited (~250-350GB/s/core
  during the 8-core burst, ~1-2us receipt latency per DMA), so groups
  taper up [256, 512, 1024...] and groups 0-1 are processed in
  m-halves: m0..3 of both groups run on just ws[0:4]+x while ws[4:8]
  still streams in (each ec-chunk's square is independent; only the
  final Exp needs all 8), keeping the PE continuously busy from ~10us.
- x for groups 0-1 is host-packed per-partition-contiguous (feath) so
  those DMAs move 4KB descriptors, not gw*4B strided runs.
- Weight pair-DMAs spread across the three rings (sync/gpsimd HWDGE/
  SWDGE); steady-state x groups alternate sync/gpsimd with the 4-group
  xin ring hiding the latency. Tail groups [512, 512, 256] shorten the
  post-matmul drain chain.
"""

import numpy as np

import concourse.bacc as bacc_mod
import concourse.mybir as mybir
import concourse.tile as tile
from concourse.bass_utils import run_bass_kernel_spmd

dt = mybir.dt
Act = mybir.ActivationFunctionType

B, F, H, W = 8, 512, 128, 128
E, C = 16, 64
PIX = H * W           # 16384 pixels per image
MCH = (E * C) // 128  # 8 ec-chunks of 128 partitions
KCH = F // 128        # 4 contraction chunks
LENGTH_SCALE = 0.1
GAMMA = -1.0 / (2.0 * LENGTH_SCALE**2)   # -50.0
EXP_SCALE = GAMMA / E                    # -3.125

GROUPS = [256, 512] + [1024] * 14 + [512, 512, 256]
assert sum(GROUPS) == PIX
assert all(g >= 256 for g in GROUPS)  # fp32r is 1/4-rate below 256 cols
GW = 1024  # allocation width; narrower groups use [:, :gw] slices
HEADW = KCH * (GROUPS[0] + GROUPS[1])


def _build():
    nc = bacc_mod.Bacc(None)
    feat_d = nc.declare_dram_parameter("feat", [F, PIX], dt.float32r, isOutput=False)
    # head-packed x for groups 0-1: per-partition contiguous 4KB+ runs
    feat_h = nc.declare_dram_parameter(
        "feath", [128, HEADW], dt.float32r, isOutput=False
    )
    wt_d = nc.declare_dram_parameter(
        "wt", [128, MCH, KCH, 128], dt.float32r, isOutput=False
    )  # partition-major: weight loads are contiguous per partition
    negc_d = nc.declare_dram_parameter("negc", [128, MCH], dt.float32, isOutput=False)
    out_d = nc.declare_dram_parameter("out", [C, PIX], dt.float32, isOutput=True)

    feat_k = feat_d.rearrange("(k p) x -> p k x", k=KCH)

    with tile.TileContext(nc) as tc:
        with (
            tc.tile_pool(name="singles", bufs=1) as singles,
            tc.tile_pool(name="xin", bufs=4) as xin,
            tc.tile_pool(name="sqp", bufs=4) as sqp,
            tc.tile_pool(name="accp", bufs=2) as accp,
            tc.tile_pool(name="outp", bufs=2) as outp,
            tc.tile_pool(name="headp", bufs=1) as headp,
            tc.tile_pool(name="ps", bufs=4, space="PSUM") as ps,
        ):
            negc_sb = singles.tile([128, MCH], dt.float32, tag="negc")

            ws_all = singles.tile([128, MCH, KCH, 128], dt.float32r, tag="ws")

            g0w, g1w = GROUPS[0], GROUPS[1]
            xh0 = headp.tile([128, KCH * g0w], dt.float32r, tag="xh0")
            xh1 = headp.tile([128, KCH * g1w], dt.float32r, tag="xh1")
            # startup wave: scalar ring: head x; sync ring: ws[0:4] as two
            # pairs; gpsimd ring: negc (above) + ws[4:8] as two pairs
            # supply-matched order (~185GB/s/core effective during the
            # 8-core burst): tiny first-MM gate (xh0 + ws0), then each
            # weight chunk lands just before the m-major schedule needs it
            nc.scalar.dma_start(out=xh0, in_=feat_h[:, 0 : KCH * g0w])
            nc.sync.dma_start(out=ws_all[:, 0:1, :, :], in_=wt_d[:, 0:1, :, :])
            nc.gpsimd.dma_start(out=ws_all[:, 1:2, :, :], in_=wt_d[:, 1:2, :, :])
            nc.sync.dma_start(out=xh1, in_=feat_h[:, KCH * g0w : HEADW])
            nc.gpsimd.dma_start(out=negc_sb, in_=negc_d[:, :])
            nc.sync.dma_start(out=ws_all[:, 2:4, :, :], in_=wt_d[:, 2:4, :, :])
            nc.sync.dma_start(out=ws_all[:, 4:6, :, :], in_=wt_d[:, 4:6, :, :])
            nc.gpsimd.dma_start(out=ws_all[:, 6:8, :, :], in_=wt_d[:, 6:8, :, :])

            accs = {}

            def half(g, xg, ms):
                """MMs + square/accumulate for ec-chunks `ms` of group g."""
                gw = GROUPS[g]
                segs = [
                    slice(t * 512, min((t + 1) * 512, gw))
                    for t in range((gw + 511) // 512)
                ]
                for m in ms:
                    pst = ps.tile([128, GW], dt.float32, tag="mm")
                    for k in range(KCH):
                        for cs in segs:
                            if g <= 1:  # head tiles are [128, KCH*gw] flat
                                rhs = xg[:, k * gw + cs.start : k * gw + cs.stop]
                            else:
                                rhs = xg[:, k, cs]
                            nc.tensor.matmul(
                                out=pst[:, cs], lhsT=ws_all[:, m, k, :],
                                rhs=rhs,
                                start=(k == 0), stop=(k == KCH - 1),
                            )
                    if m == 0:
                        accs[g] = accp.tile([128, GW], dt.float32, tag="acc", name="acc")
                        nc.scalar.activation(
                            out=accs[g][:, 0:gw], in_=pst[:, 0:gw],
                            func=Act.Square, bias=negc_sb[:, 0:1], scale=1.0,
                        )
                    else:
                        sq = sqp.tile([128, GW], dt.float32, tag="sq")
                        nc.scalar.activation(
                            out=sq[:, 0:gw], in_=pst[:, 0:gw], func=Act.Square,
                            bias=negc_sb[:, m : m + 1], scale=1.0,
                        )
                        nc.vector.tensor_add(
                            out=accs[g][:, 0:gw], in0=accs[g][:, 0:gw],
                            in1=sq[:, 0:gw],
                        )

            def finish(g, px):
                gw = GROUPS[g]
                acc = accs.pop(g)
                tmp = outp.tile([64, GW], dt.float32, tag="tmp")
                nc.vector.tensor_copy(out=tmp[:, 0:gw], in_=acc[64:128, 0:gw])
                hc = outp.tile([64, GW], dt.float32, tag="hc")
                nc.vector.tensor_add(
                    out=hc[:, 0:gw], in0=acc[0:64, 0:gw], in1=tmp[:, 0:gw]
                )
                eo = outp.tile([64, GW], dt.float32, tag="eo")
                nc.scalar.activation(
                    out=eo[:, 0:gw], in_=hc[:, 0:gw], func=Act.Exp,
                    bias=0.0, scale=EXP_SCALE,
                )
                nc.scalar.dma_start(out=out_d[:, px], in_=eo[:, 0:gw])

            # groups 0-1 in m-halves: PE runs on ws[0:4] while ws[4:8] lands
            px_g = [slice(sum(GROUPS[:g]), sum(GROUPS[: g + 1]))
                    for g in range(len(GROUPS))]
            half(0, xh0, [0])
            half(0, xh0, [1])
            half(1, xh1, [0, 1])
            half(0, xh0, [2, 3])
            half(1, xh1, [2, 3])
            half(0, xh0, [4, 5, 6, 7])
            finish(0, px_g[0])
            half(1, xh1, [4, 5, 6, 7])
            finish(1, px_g[1])

            for g in range(2, len(GROUPS)):
                gw = GROUPS[g]
                xg = xin.tile([128, KCH, GW], dt.float32r, tag="x")
                if g == 2:
                    q = nc.scalar
                else:
                    q = nc.sync if g % 2 == 0 else nc.gpsimd
                q.dma_start(out=xg[:, 0:2, 0:gw], in_=feat_k[:, 0:2, px_g[g]])
                q.dma_start(out=xg[:, 2:4, 0:gw], in_=feat_k[:, 2:4, px_g[g]])
                half(g, xg, range(MCH))
                finish(g, px_g[g])

    nc.finalize()
    return nc


_NC_CACHE = {}


def _get_nc():
    if "nc" not in _NC_CACHE:
        _NC_CACHE["nc"] = _build()
    return _NC_CACHE["nc"]


def _prep_inputs(features, weights, m, N):
    # wtp[p, mc, k, j] = W_T[f = k*128 + p, col = mc*128 + j] where
    # W_T[f, e*64+c] = weights[e, c, f]: each weight pair-DMA is a
    # contiguous run per partition.
    wtf = weights.astype(np.float32).transpose(2, 0, 1).reshape(F, E * C)
    wt = np.ascontiguousarray(
        wtf.reshape(KCH, 128, MCH, 128).transpose(1, 2, 0, 3)
    )
    cent = (m.astype(np.float32) / N.astype(np.float32)[None, :]).reshape(-1)  # [ec]
    negc = np.ascontiguousarray(-cent.reshape(MCH, 128).T)  # [128, MCH]
    feats = np.ascontiguousarray(features.astype(np.float32).reshape(B, F, PIX))
    # head-packed x for groups 0-1: head[b, p, :] = [k0 g0px | ... | k3 g0px
    # | k0 g1px | ... ] so each head DMA is one contiguous run per partition
    f4 = feats.reshape(B, KCH, 128, PIX)
    g0w, g1w = GROUPS[0], GROUPS[1]
    head = np.concatenate(
        [
            f4[:, :, :, 0:g0w].transpose(0, 2, 1, 3).reshape(B, 128, -1),
            f4[:, :, :, g0w : g0w + g1w].transpose(0, 2, 1, 3).reshape(B, 128, -1),
        ],
        axis=2,
    )
    head = np.ascontiguousarray(head)
    return [
        {"feat": feats[i], "feath": head[i], "wt": wt, "negc": negc}
        for i in range(B)
    ]


def run_spmd(features, weights, m, N, trace=False):
    in_maps = _prep_inputs(features, weights, m, N)
    res = run_bass_kernel_spmd(_get_nc(), in_maps, list(range(B)), trace=trace)
    out = np.stack([res.results[i]["out"] for i in range(B)])  # [B, C, PIX]
    return out.reshape(B, C, H, W).astype(np.float32), res


def kernel(features, weights, m, N):
    out, _ = run_spmd(features, weights, m, N, trace=False)
    return out


# revision 16
# speedup vs baseline: 1.0393x; 1.0393x over previous
"""DUQ RBF head kernel for Trainium2 (8 NeuronCores, batch-parallel).

Computes out[b,c,h,w] = exp(gamma * mean_e (einsum('bfhw,ecf', x, W) - m/N)^2)
for features [8,512,128,128], weights [16,64,512], m [16,64], N [64].

Strategy: data-parallel over batch (1 image per core). Per core, one big
matmul [ec=1024, f=512] @ [f=512, pix=16384] in float32r (full-rate fp32,
self-loading weights; LDWEIGHTS hides behind the previous matmul's
streaming; every matmul carries a fixed ~13.5ns issue overhead on top of
N/2.4GHz streaming, so N=512 everywhere it can be). Pixels are processed
in groups of up to 1024 (two 512-col PSUM banks per ec-chunk); the Square
epilogue folds the centroid into the per-partition ACT bias, DVE
accumulates the 8 squared ec-chunks and folds the two e-halves, ACT
applies the final Exp. N per matmul stays >= 256: fp32r is 1/4-rate
below 256 moving columns.

Startup (the schedulable slack; steady state is PE-streaming-bound):
- The first ~2MB of weights+x is HBM/ring-limited (~250-350GB/s/core
  during the 8-core burst, ~1-2us receipt latency per DMA), so groups
  taper up [256, 512, 1024...] and groups 0-1 are processed in
  m-halves: m0..3 of both groups run on just ws[0:4]+x while ws[4:8]
  still streams in (each ec-chunk's square is independent; only the
  final Exp needs all 8), keeping the PE continuously busy from ~10us.
- x for groups 0-1 is host-packed per-partition-contiguous (feath) so
  those DMAs move 4KB descriptors, not gw*4B strided runs.
- Weight pair-DMAs spread across the three rings (sync/gpsimd HWDGE/
  SWDGE); steady-state x groups alternate sync/gpsimd with the 4-group
  xin ring hiding the latency. Tail groups [512, 512, 256] shorten the
  post-matmul drain chain.
"""

import numpy as np

import concourse.bacc as bacc_mod
import concourse.mybir as mybir
import concourse.tile as tile
from concourse.bass_utils import run_bass_kernel_spmd

dt = mybir.dt
Act = mybir.ActivationFunctionType

B, F, H, W = 8, 512, 128, 128
E, C = 16, 64
PIX = H * W           # 16384 pixels per image
MCH = (E * C) // 128  # 8 ec-chunks of 128 partitions
KCH = F // 128        # 4 contraction chunks
LENGTH_SCALE = 0.1
GAMMA = -1.0 / (2.0 * LENGTH_SCALE**2)   # -50.0
EXP_SCALE = GAMMA / E                    # -3.125

GROUPS = [512, 512] + [1024] * 14 + [512, 512]
assert sum(GROUPS) == PIX
assert all(g >= 256 for g in GROUPS)  # fp32r is 1/4-rate below 256 cols
GW = 1024  # allocation width; narrower groups use [:, :gw] slices
HEADW = KCH * (GROUPS[0] + GROUPS[1])


def _build():
    nc = bacc_mod.Bacc(None)
    feat_d = nc.declare_dram_parameter("feat", [F, PIX], dt.float32r, isOutput=False)
    # head-packed x for groups 0-1: per-partition contiguous 4KB+ runs
    feat_h = nc.declare_dram_parameter(
        "feath", [128, HEADW], dt.float32r, isOutput=False
    )
    wt_d = nc.declare_dram_parameter(
        "wt", [128, MCH, KCH, 128], dt.float32r, isOutput=False
    )  # partition-major: weight loads are contiguous per partition
    negc_d = nc.declare_dram_parameter("negc", [128, MCH], dt.float32, isOutput=False)
    out_d = nc.declare_dram_parameter("out", [C, PIX], dt.float32, isOutput=True)

    feat_k = feat_d.rearrange("(k p) x -> p k x", k=KCH)

    with tile.TileContext(nc) as tc:
        with (
            tc.tile_pool(name="singles", bufs=1) as singles,
            tc.tile_pool(name="xin", bufs=4) as xin,
            tc.tile_pool(name="sqp", bufs=4) as sqp,
            tc.tile_pool(name="accp", bufs=2) as accp,
            tc.tile_pool(name="outp", bufs=2) as outp,
            tc.tile_pool(name="headp", bufs=1) as headp,
            tc.tile_pool(name="ps", bufs=4, space="PSUM") as ps,
        ):
            negc_sb = singles.tile([128, MCH], dt.float32, tag="negc")

            ws_all = singles.tile([128, MCH, KCH, 128], dt.float32r, tag="ws")

            g0w, g1w = GROUPS[0], GROUPS[1]
            xh0 = headp.tile([128, KCH * g0w], dt.float32r, tag="xh0")
            xh1 = headp.tile([128, KCH * g1w], dt.float32r, tag="xh1")
            # startup wave: scalar ring: head x; sync ring: ws[0:4] as two
            # pairs; gpsimd ring: negc (above) + ws[4:8] as two pairs
            # supply-matched order (~185GB/s/core effective during the
            # 8-core burst): tiny first-MM gate (xh0 + ws0), then each
            # weight chunk lands just before the m-major schedule needs it
            nc.scalar.dma_start(out=xh0, in_=feat_h[:, 0 : KCH * g0w])
            nc.sync.dma_start(out=ws_all[:, 0:1, :, :], in_=wt_d[:, 0:1, :, :])
            nc.gpsimd.dma_start(out=ws_all[:, 1:2, :, :], in_=wt_d[:, 1:2, :, :])
            nc.sync.dma_start(out=xh1, in_=feat_h[:, KCH * g0w : HEADW])
            nc.gpsimd.dma_start(out=negc_sb, in_=negc_d[:, :])
            nc.sync.dma_start(out=ws_all[:, 2:4, :, :], in_=wt_d[:, 2:4, :, :])
            nc.gpsimd.dma_start(out=ws_all[:, 4:6, :, :], in_=wt_d[:, 4:6, :, :])
            nc.gpsimd.dma_start(out=ws_all[:, 6:8, :, :], in_=wt_d[:, 6:8, :, :])

            accs = {}

            def half(g, xg, ms):
                """MMs + square/accumulate for ec-chunks `ms` of group g."""
                gw = GROUPS[g]
                segs = [
                    slice(t * 512, min((t + 1) * 512, gw))
                    for t in range((gw + 511) // 512)
                ]
                for m in ms:
                    pst = ps.tile([128, GW], dt.float32, tag="mm")
                    for k in range(KCH):
                        for cs in segs:
                            if g <= 1:  # head tiles are [128, KCH*gw] flat
                                rhs = xg[:, k * gw + cs.start : k * gw + cs.stop]
                            else:
                                rhs = xg[:, k, cs]
                            nc.tensor.matmul(
                                out=pst[:, cs], lhsT=ws_all[:, m, k, :],
                                rhs=rhs,
                                start=(k == 0), stop=(k == KCH - 1),
                            )
                    if m == 0:
                        accs[g] = accp.tile([128, GW], dt.float32, tag="acc", name="acc")
                        nc.scalar.activation(
                            out=accs[g][:, 0:gw], in_=pst[:, 0:gw],
                            func=Act.Square, bias=negc_sb[:, 0:1], scale=1.0,
                        )
                    else:
                        sq = sqp.tile([128, GW], dt.float32, tag="sq")
                        nc.scalar.activation(
                            out=sq[:, 0:gw], in_=pst[:, 0:gw], func=Act.Square,
                            bias=negc_sb[:, m : m + 1], scale=1.0,
                        )
                        nc.vector.tensor_add(
                            out=accs[g][:, 0:gw], in0=accs[g][:, 0:gw],
                            in1=sq[:, 0:gw],
                        )

            def finish(g, px):
                gw = GROUPS[g]
                acc = accs.pop(g)
                tmp = outp.tile([64, GW], dt.float32, tag="tmp")
                nc.vector.tensor_copy(out=tmp[:, 0:gw], in_=acc[64:128, 0:gw])
                hc = outp.tile([64, GW], dt.float32, tag="hc")
                nc.vector.tensor_add(
                    out=hc[:, 0:gw], in0=acc[0:64, 0:gw], in1=tmp[:, 0:gw]
                )
                eo = outp.tile([64, GW], dt.float32, tag="eo")
                nc.scalar.activation(
                    out=eo[:, 0:gw], in_=hc[:, 0:gw], func=Act.Exp,
                    bias=0.0, scale=EXP_SCALE,
                )
                nc.scalar.dma_start(out=out_d[:, px], in_=eo[:, 0:gw])

            # groups 0-1 in m-halves: PE runs on ws[0:4] while ws[4:8] lands
            px_g = [slice(sum(GROUPS[:g]), sum(GROUPS[: g + 1]))
                    for g in range(len(GROUPS))]
            half(0, xh0, [0])
            half(0, xh0, [1])
            half(1, xh1, [0, 1])
            half(0, xh0, [2, 3])
            half(1, xh1, [2, 3])
            half(0, xh0, [4, 5])
            half(1, xh1, [4, 5])
            half(0, xh0, [6, 7])
            finish(0, px_g[0])
            half(1, xh1, [6, 7])
            finish(1, px_g[1])

            for g in range(2, len(GROUPS)):
                gw = GROUPS[g]
                xg = xin.tile([128, KCH, GW], dt.float32r, tag="x")
                q = nc.sync if g % 2 == 0 else nc.gpsimd
                q.dma_start(out=xg[:, 0:2, 0:gw], in_=feat_k[:, 0:2, px_g[g]])
                q.dma_start(out=xg[:, 2:4, 0:gw], in_=feat_k[:, 2:4, px_g[g]])
                half(g, xg, range(MCH))
                finish(g, px_g[g])

    nc.finalize()
    return nc


_NC_CACHE = {}


def _get_nc():
    if "nc" not in _NC_CACHE:
        _NC_CACHE["nc"] = _build()
    return _NC_CACHE["nc"]


def _prep_inputs(features, weights, m, N):
    # wtp[p, mc, k, j] = W_T[f = k*128 + p, col = mc*128 + j] where
    # W_T[f, e*64+c] = weights[e, c, f]: each weight pair-DMA is a
    # contiguous run per partition.
    wtf = weights.astype(np.float32).transpose(2, 0, 1).reshape(F, E * C)
    wt = np.ascontiguousarray(
        wtf.reshape(KCH, 128, MCH, 128).transpose(1, 2, 0, 3)
    )
    cent = (m.astype(np.float32) / N.astype(np.float32)[None, :]).reshape(-1)  # [ec]
    negc = np.ascontiguousarray(-cent.reshape(MCH, 128).T)  # [128, MCH]
    feats = np.ascontiguousarray(features.astype(np.float32).reshape(B, F, PIX))
    # head-packed x for groups 0-1: head[b, p, :] = [k0 g0px | ... | k3 g0px
    # | k0 g1px | ... ] so each head DMA is one contiguous run per partition
    f4 = feats.reshape(B, KCH, 128, PIX)
    g0w, g1w = GROUPS[0], GROUPS[1]
    head = np.concatenate(
        [
            f4[:, :, :, 0:g0w].transpose(0, 2, 1, 3).reshape(B, 128, -1),
            f4[:, :, :, g0w : g0w + g1w].transpose(0, 2, 1, 3).reshape(B, 128, -1),
        ],
        axis=2,
    )
    head = np.ascontiguousarray(head)
    return [
        {"feat": feats[i], "feath": head[i], "wt": wt, "negc": negc}
        for i in range(B)
    ]


def run_spmd(features, weights, m, N, trace=False):
    in_maps = _prep_inputs(features, weights, m, N)
    res = run_bass_kernel_spmd(_get_nc(), in_maps, list(range(B)), trace=trace)
    out = np.stack([res.results[i]["out"] for i in range(B)])  # [B, C, PIX]
    return out.reshape(B, C, H, W).astype(np.float32), res


def kernel(features, weights, m, N):
    out, _ = run_spmd(features, weights, m, N, trace=False)
    return out
